# revision 1
# baseline (speedup 1.0000x reference)
"""AffinityPropagate Trainium2 kernel.

Math (per batch image, reference semantics):
    w_k = |a_k| / sum_k |a_k|            (per-pixel, 9 taps, k=(dy,dx))
    f <- sum_k w_k * shift_k(pad0(f))    repeated 4 times

Sharding: pure data parallel — batch 8 -> 8 NeuronCores, one image each.

Layout per core (flat-chunk):
    The image is flattened to q = y*W + x in [0, H*W); partition p owns the
    contiguous pixel chunk [p*CH, (p+1)*CH), CH = H*W/128 = 4080.  The feature
    buffer [128, CH + 2*HA] stores each chunk with HA = W+1 halo pixels
    duplicated on both sides, so every 3x3 tap is a free-dim offset
    off = dy*W + dx; no compute op ever needs a partition offset (HW requires
    quadrant-aligned partition starts).  Halos are refreshed after each
    iteration on TensorE with constant shift matrices (their zero rows keep
    the outermost halos at exactly 0 = the reference's dy zero padding).

    In flat indexing, a dx=-1 tap at x=0 wraps to the previous row's last
    pixel (and dx=+1 at x=W-1 to the next row's first), where the reference
    sees zero padding.  Since padding only zeroes the *feature* read (the
    denominator sum_k |a_k| still counts every tap), this is exactly
    equivalent to zeroing those taps' weights at the wrap columns; the
    column masks arrive as a constant fp16 input.

    Engine split:
      ScalarE   |a| fp32->fp16 convert, halo evac, shifted f2 copy
      TensorE   channel-sum of |a|, 9-tap product accumulation into PSUM
                (identity matmuls, start/stop groups), halo partition shifts
      VectorE   fp16 2x-mode tap products; 1/sum (approx); psum * r evac
      DMA       coarse 3D loads only (HWDGE via both SP and ACT queues)
    The feature lives in fp16; a one-element-shifted copy f2 keeps every tap
    product 4-byte aligned so the DVE's fp16 2x mode engages (dx=0 taps have
    odd flat offsets and read f2 instead).

    Schedule: the 18.8MB fp32 affinity read is the serial HBM resource, so
    iteration 0 is cut into 1020-px chunks interleaved into the
    normalization stream as each weight range completes — the DVE computes
    under the DMA stream and iterations 1-3 then run DVE-bound.
    Weight magnitudes stay unnormalized (|a| in fp16); 1/sum is folded into
    the PSUM evacuation multiply each iteration, avoiding a 9-plane rescale.
"""

import numpy as np

import concourse.bacc as bacc
import concourse.bass as bass
import concourse.mybir as mybir
import concourse.tile as tile
from concourse.bass_utils import run_bass_kernel_spmd

H, W = 544, 960
NPIX = H * W
NK = 9
CH = NPIX // 128  # 4080 pixels per partition
HA = W + 1  # halo on each side
FW = CH + 2 * HA  # feature row length per partition
ITERS = 4
CW = 255  # norm column chunk (16 chunks)
CI = 2040  # iteration chunk (2 chunks, 4 PSUM banks each)
OFFS = [dy * W + dx for dy in (-1, 0, 1) for dx in (-1, 0, 1)]
# per-chunk tap emission order: taps that only read interior data first,
# halo-dependent next, f2(odd-offset)-dependent last — lets the scheduler
# start next-iteration products before the halo/f2 refresh completes.
KORDER = [5, 6, 8, 0, 2, 3, 1, 4, 7]
AF = mybir.AluOpType
DT = mybir.dt
F16 = DT.float16
F32 = DT.float32

_nc_cache = {}


def _build():
    nc = bacc.Bacc(
        "TRN2",
        target_bir_lowering=False,
        debug=False,
        enable_asserts=False,
    )
    a = nc.dram_tensor("a", [NK, H, W], F32, kind="ExternalInput").ap()
    f = nc.dram_tensor("f", [H, W], F32, kind="ExternalInput").ap()
    m = nc.dram_tensor("m", [128, 2, 2 * W], F16, kind="ExternalInput").ap()
    ident = nc.dram_tensor("ident", [128, 3, 128], F16, kind="ExternalInput").ap()
    o = nc.dram_tensor("o", [H, W], F32, kind="ExternalOutput").ap()

    with tile.TileContext(nc) as tc:
        _build_tile(tc, a, f, m, ident, o)
    nc.finalize()
    return nc


def _expand(sl, step, n):
    """Insert a [step, n] dim after the partition dim of a 2D AP slice."""
    return bass.AP(
        tensor=sl.tensor, offset=sl.offset, ap=[sl.ap[0], [step, n], *sl.ap[1:]]
    )


def _build_tile(tc, a, f, m, ident, o):
    nc = tc.nc
    # flattened per-partition views of the DRAM tensors
    av = (
        a.rearrange("k h w -> k (h w)")
        .rearrange("k (p j) -> k p j", p=128)
        .rearrange("k p j -> p k j")
    )
    ff = f.rearrange("h w -> (h w)").rearrange("(p j) -> p j", p=128)
    of = o.rearrange("h w -> (h w)").rearrange("(p j) -> p j", p=128)

    with (
        tc.tile_pool(name="persist", bufs=1) as persist,
        tc.tile_pool(name="stage", bufs=3) as stage_pool,
        tc.tile_pool(name="prodp", bufs=3) as prodp,
        tc.tile_pool(name="outp", bufs=2) as outp,
        tc.tile_pool(name="psum", bufs=2, space="PSUM") as psump,
    ):
        fb = [persist.tile([128, FW], F16, name=f"f{i}") for i in range(2)]
        f2 = persist.tile([128, FW], F16, name="f2")
        aw = persist.tile([128, NK, CH], F16, name="aw")
        r = persist.tile([128, CH], F32, name="r")
        # wrap-column masks, one 2*W-periodic row per partition phase
        # (partition p starts at pixel 4080p; 4080 mod 960 = 240 -> 4 phases)
        msk = persist.tile([128, 2, 2 * W], F16, name="msk")
        idt3 = persist.tile([128, 3, 128], F16, name="idt3")

        def load_consts():
            # plain contiguous copies — the host pre-expands both tables to
            # their per-partition SBUF layouts
            nc.sync.dma_start(out=idt3[:], in_=ident)
            nc.scalar.dma_start(out=msk[:], in_=m)

        idt = idt3[:, 0, :]
        sdn = idt3[:, 1, :]
        sup = idt3[:, 2, :]

        def apply_masks(c0, cw):
            # zero the row-wrap taps in ONE op: viewing k=(dy,dx), the dx=-1
            # planes (k%3==0) get the x==0 mask and dx=+1 planes (k%3==2) the
            # x==W-1 mask; dx in {0,2} is a step-2 slice and the two mask rows
            # are adjacent in msk.  msk is W-periodic; cw<=W.
            aw4 = aw[:].rearrange("p (dy dx) c -> p dy dx c", dy=3)[
                :, :, 0::2, c0 : c0 + cw
            ]
            mt = msk[:, 0, c0 % W : c0 % W + cw]
            m4 = bass.AP(
                tensor=mt.tensor,
                offset=mt.offset,
                ap=[mt.ap[0], [0, 3], [2 * W, 2], *mt.ap[1:]],
            )
            nc.vector.tensor_mul(out=aw4, in0=aw4, in1=m4)

        def norm_chunk(ci, c0):
            st = stage_pool.tile([128, NK, CW], F32, name="st", tag="st")
            dmae = nc.sync if ci % 2 == 0 else nc.scalar
            dmae.dma_start(out=st[:], in_=av[:, :, c0 : c0 + CW])
            awc = aw[:, :, c0 : c0 + CW]
            nc.scalar.activation(
                out=awc, in_=st[:], func=mybir.ActivationFunctionType.Abs
            )
            s = psump.tile([128, CW], F32, name="s", tag="acc")
            for k in range(NK):
                nc.tensor.matmul(
                    s[:],
                    idt[:],
                    aw[:, k, c0 : c0 + CW],
                    start=(k == 0),
                    stop=(k == NK - 1),
                )
            nc.vector.reciprocal_approx_fast(out=r[:, c0 : c0 + CW], in_=s[:])
            apply_masks(c0, CW)

        def iter_chunk(t, c0, ci=CI):
            fc, fn = fb[t % 2], fb[(t + 1) % 2]
            last = t == ITERS - 1
            acc = psump.tile([128, ci], F32, name="acc", tag="acc")
            for ki, k in enumerate(KORDER):
                base = HA + OFFS[k] + c0
                if base % 2:  # odd fp16 offset: read the shifted copy
                    src = f2[:, base - 1 : base - 1 + ci]
                else:
                    src = fc[:, base : base + ci]
                prod = prodp.tile([128, ci], F16, name="prod", tag="prod")
                nc.vector.tensor_mul(out=prod[:], in0=aw[:, k, c0 : c0 + ci], in1=src)
                for s0 in range(0, ci, 512):
                    se = min(s0 + 512, ci)
                    nc.tensor.matmul(
                        acc[:, s0:se],
                        idt[:],
                        prod[:, s0:se],
                        start=(ki == 0),
                        stop=(ki == NK - 1),
                    )
            rc = r[:, c0 : c0 + ci]
            if last:
                ost = outp.tile([128, ci], F32, name="ost", tag="ost")
                if c0 + ci == CH:  # final chunk: slice so the DMA tail overlaps
                    for q0 in range(0, ci, 510):
                        nc.vector.tensor_mul(
                            out=ost[:, q0 : q0 + 510],
                            in0=acc[:, q0 : q0 + 510],
                            in1=r[:, c0 + q0 : c0 + q0 + 510],
                        )
                        nc.sync.dma_start(
                            out=of[:, c0 + q0 : c0 + q0 + 510],
                            in_=ost[:, q0 : q0 + 510],
                        )
                else:
                    nc.vector.tensor_mul(out=ost[:], in0=acc[:], in1=rc)
                    nc.sync.dma_start(out=of[:, c0 : c0 + ci], in_=ost[:])
            else:
                nc.vector.tensor_mul(
                    out=fn[:, HA + c0 : HA + c0 + ci], in0=acc[:], in1=rc
                )

        # ---- schedule ----
        # The HBM stream (weights 18.8MB + feature/masks) is the serial
        # resource during the first phase; iteration-0 work is cut into
        # 1020-px chunks and interleaved into the normalization stream as
        # soon as each aw/r range is ready, filling the DVE under the DMAs.
        ncw = CH // CW
        load_consts()
        for ci in range(4):
            norm_chunk(ci, ci * CW)
        for c0 in range(0, CH, 2040):
            fst = outp.tile([128, 2040], F32, name="fst", tag="ost")
            nc.sync.dma_start(out=fst[:], in_=ff[:, c0 : c0 + 2040])
            nc.vector.tensor_copy(out=fb[0][:, HA + c0 : HA + c0 + 2040], in_=fst[:])
        _refresh(nc, psump, fb[0], f2, sdn, sup)

        nq = 1020 // CW  # norm chunks per iter0 chunk
        iter_chunk(0, 0, 1020)
        for ci in range(4, ncw):
            norm_chunk(ci, ci * CW)
            if (ci + 1) % nq == 0 and (ci + 1) // nq <= 4:
                iter_chunk(0, ((ci + 1) // nq - 1) * 1020, 1020)
        _refresh(nc, psump, fb[1], f2, sdn, sup)

        for t in range(1, ITERS):
            for c0 in range(0, CH, CI):
                iter_chunk(t, c0)
            if t != ITERS - 1:
                _refresh(nc, psump, fb[(t + 1) % 2], f2, sdn, sup)


def _refresh(nc, psump, ft, f2, sdn, sup):
    """Halo exchange (partition shift on TensorE) + rebuild the shifted copy.

    sdn[k,m]=1 iff m=k+1 so psum[p] = rhs[p-1] (row 0 -> 0); sup shifts the
    other way (row 127 -> 0).  The zero rows keep the outermost halos at
    exactly 0, which implements the dy zero padding of the reference."""
    # 1024-float tiles so every matmul lands bank-aligned
    phR = psump.tile([128, 1024], F32, name="phR", tag="acc")
    phL = psump.tile([128, 1024], F32, name="phL", tag="acc")
    # right halo first: it reads the first chunk's data, which is ready earlier
    for s0 in range(0, HA, 512):
        se = min(s0 + 512, HA)
        nc.tensor.matmul(
            phR[:, s0:se], sup, ft[:, HA + s0 : HA + se], start=True, stop=True
        )
    nc.scalar.copy(out=ft[:, HA + CH : FW], in_=phR[:, 0:HA])
    for s0 in range(0, HA, 512):
        se = min(s0 + 512, HA)
        nc.tensor.matmul(
            phL[:, s0:se], sdn, ft[:, CH + s0 : CH + se], start=True, stop=True
        )
    nc.scalar.copy(out=ft[:, 0:HA], in_=phL[:, 0:HA])
    nc.scalar.copy(out=f2[:, 0 : FW - 1], in_=ft[:, 1:FW])


def _masks():
    # msk[p, mi, col] = mask value at pixel x = (240*(p%4) + col) mod W —
    # partition p starts at pixel 4080p and 4080 mod W = 240, so the
    # W-periodic wrap-column masks have 4 partition phases
    col = np.arange(2 * W)
    out = np.empty((128, 2, 2 * W), np.float16)
    for ph in range(4):
        x = (240 * ph + col) % W
        out[ph::4, 0] = (x != 0).astype(np.float16)
        out[ph::4, 1] = (x != W - 1).astype(np.float16)
    return out


def _get_nc():
    if "nc" not in _nc_cache:
        _nc_cache["nc"] = _build()
    return _nc_cache["nc"]


def _run(affinity, feature, **spmd_kwargs):
    affinity = np.ascontiguousarray(np.asarray(affinity, dtype=np.float32))
    feature = np.ascontiguousarray(np.asarray(feature, dtype=np.float32))
    nbatch = affinity.shape[0]
    nc = _get_nc()
    masks = _masks()
    ident = np.ascontiguousarray(
        np.stack(
            [
                np.eye(128, dtype=np.float16),
                np.eye(128, k=1, dtype=np.float16),
                np.eye(128, k=-1, dtype=np.float16),
            ]
        ).transpose(1, 0, 2)
    )
    in_maps = [
        {"a": affinity[i], "f": feature[i, 0], "m": masks, "ident": ident}
        for i in range(nbatch)
    ]
    res = run_bass_kernel_spmd(nc, in_maps, core_ids=list(range(nbatch)), **spmd_kwargs)
    out = np.stack([r["o"] for r in res.results])[:, None, :, :]
    return out.astype(np.float32), res


def kernel(affinity, feature):
    out, _ = _run(affinity, feature)
    return out



# revision 6
# speedup vs baseline: 1.0173x; 1.0173x over previous
"""AffinityPropagate Trainium2 kernel.

Math (per batch image, reference semantics):
    w_k = |a_k| / sum_k |a_k|            (per-pixel, 9 taps, k=(dy,dx))
    f <- sum_k w_k * shift_k(pad0(f))    repeated 4 times

Sharding: pure data parallel - batch 8 -> 8 NeuronCores, one image each.

Layout per core (flat-chunk):
    The image is flattened to q = y*W + x in [0, H*W); partition p owns the
    contiguous pixel chunk [p*CH, (p+1)*CH), CH = H*W/128 = 4080.  The feature
    buffer [128, CH + 2*HA] stores each chunk with HA = W+1 halo pixels
    duplicated on both sides, so every 3x3 tap is a free-dim offset
    off = dy*W + dx.

    In flat indexing, a dx=-1 tap at x=0 wraps to the previous row's last
    pixel (and dx=+1 at x=W-1 to the next row's first), where the reference
    sees zero padding.  Since padding only zeroes the *feature* read (the
    denominator sum_k |a_k| still counts every tap), this is equivalent to
    zeroing those taps' weights at the wrap columns.

    Normalization is folded into the weights once: w = |a| * mask * (1/sum),
    computed chunk-by-chunk as the affinity stream arrives (one fused
    [128,3,3,cw] multiply against rm = mask3 * r16).  Iterations then need no
    per-pixel rescale - PSUM evacuation is a plain fp32->fp16 copy on the
    Activation engine.

    Engine split (steady-state iteration):
      DVE       ~7 tap-product planes per chunk as fused 4-dim fp16 muls (2x)
      Pool      2 tap-product planes per chunk (idle engine, slower rate)
      TensorE   9 wide identity matmuls per chunk accumulate planes in PSUM
      ScalarE   |a| converts (phase 1), PSUM evacuation copies, halo evac
      DMA       loads phase 1; halo partition-shift copies during iterations

    Halo refresh: phase-1 buffers get PE partition-shift matmuls (zero rows
    establish the outer zero padding); iteration buffers reuse those zeros and
    refresh the interior halo with partition-shifted SBUF->SBUF DMAs (the DMA
    queues are idle once the affinity stream finishes).

    Schedule: the 18.8MB fp32 affinity read is the serial HBM resource, so
    iteration 0 is cut into 1020-px chunks interleaved into the
    normalization stream as each weight range completes; iterations 1-3 run
    engine-balanced across DVE/Pool/PE with per-chunk pipelining.
"""

import numpy as np

import concourse.bacc as bacc
import concourse.bass as bass
import concourse.mybir as mybir
import concourse.tile as tile
from concourse.bass_utils import run_bass_kernel_spmd

H, W = 544, 960
NPIX = H * W
NK = 9
CH = NPIX // 128  # 4080 pixels per partition
HA = W + 1  # halo on each side
FW = CH + 2 * HA  # feature row length per partition
ITERS = 4
CW = 255  # norm column chunk (16 chunks)
CI = 1020  # iteration chunk (4 chunks)
MW = W + CW  # stored mask width (mask is W-periodic, reads start at c0 % W)
AF = mybir.AluOpType
DT = mybir.dt
F16 = DT.float16
F32 = DT.float32

# tap-product plane groups per iteration chunk: (engine, dy0, ndy, dx0, ndx)
# in index coords (0..2 ~ dy,dx = -1..+1).  Interior groups first; groups that
# read the chunk's halo side come last so the halo refresh can overlap.  The
# Pool engine gets 2 planes per chunk (1 on the last) to offload the DVE.
GROUPS = {
    0: [
        ("pool", 2, 1, 1, 2),
        ("vec", 1, 1, 1, 2),
        ("vec", 2, 1, 0, 1),
        ("vec", 0, 1, 0, 3),  # dy=-1 row: needs left halo
        ("vec", 1, 1, 0, 1),  # (0,-1): last col of left halo
    ],
    1: [
        ("pool", 2, 1, 1, 2),
        ("vec", 0, 2, 0, 3),
        ("vec", 2, 1, 0, 1),
    ],
    3: [
        ("pool", 0, 1, 0, 1),
        ("vec", 0, 1, 1, 2),
        ("vec", 1, 1, 0, 2),
        ("vec", 2, 1, 0, 3),  # dy=+1 row: needs right halo
        ("vec", 1, 1, 2, 1),  # (0,+1): first col of right halo
    ],
}
GROUPS[2] = GROUPS[1]
# matmul accumulation order per chunk (indices into GROUPS[c]): fast DVE
# products first, slow Pool plane mid, halo-dependent planes last.
MM_ORDER = {0: [1, 2, 0, 3, 4], 1: [1, 2, 0], 2: [1, 2, 0], 3: [1, 2, 0, 3, 4]}

_nc_cache = {}


def _build():
    nc = bacc.Bacc(
        "TRN2",
        target_bir_lowering=False,
        debug=False,
        enable_asserts=False,
    )
    a = nc.dram_tensor("a", [NK, H, W], F32, kind="ExternalInput").ap()
    f = nc.dram_tensor("f", [H, W], F32, kind="ExternalInput").ap()
    m = nc.dram_tensor("m", [128, 3, MW], F16, kind="ExternalInput").ap()
    ident = nc.dram_tensor("ident", [128, 3, 128], F16, kind="ExternalInput").ap()
    o = nc.dram_tensor("o", [H, W], F32, kind="ExternalOutput").ap()

    with tile.TileContext(nc) as tc:
        _build_tile(tc, a, f, m, ident, o)
    nc.finalize()
    return nc


def _bcast(sl, n):
    """Insert a [0, n] broadcast dim after the partition dim of an AP."""
    return bass.AP(
        tensor=sl.tensor, offset=sl.offset, ap=[sl.ap[0], [0, n], *sl.ap[1:]]
    )


def _build_tile(tc, a, f, m, ident, o):
    nc = tc.nc
    # flattened per-partition views of the DRAM tensors
    av = (
        a.rearrange("k h w -> k (h w)")
        .rearrange("k (p j) -> k p j", p=128)
        .rearrange("k p j -> p k j")
    )
    ff = f.rearrange("h w -> (h w)").rearrange("(p j) -> p j", p=128)
    of = o.rearrange("h w -> (h w)").rearrange("(p j) -> p j", p=128)

    with (
        tc.tile_pool(name="persist", bufs=1) as persist,
        tc.tile_pool(name="stage", bufs=2) as stage_pool,
        tc.tile_pool(name="small", bufs=2) as small,
        tc.tile_pool(name="prodp", bufs=2) as prodp,
        tc.tile_pool(name="outp", bufs=3) as outp,
        tc.tile_pool(name="psum", bufs=2, space="PSUM") as psump,
    ):
        fb = [persist.tile([128, FW], F16, name=f"f{i}") for i in range(2)]
        aw = persist.tile([128, NK, CH], F16, name="aw")
        msk3 = persist.tile([128, 3, MW], F16, name="msk3")
        idt3 = persist.tile([128, 3, 128], F16, name="idt3")

        idt = idt3[:, 0, :]
        sdn = idt3[:, 1, :]
        sup = idt3[:, 2, :]

        def aw4(dy0, ndy, dx0, ndx, c0, cw):
            return aw[:].rearrange("p (dy dx) c -> p dy dx c", dy=3)[
                :, dy0 : dy0 + ndy, dx0 : dx0 + ndx, c0 : c0 + cw
            ]

        def fview(ft, base, dy0, ndy, dx0, ndx, cw):
            """[128, ndy, ndx, cw] view of ft at tap offsets dy*W + dx."""
            sl = ft[:, 0:cw]
            return bass.AP(
                tensor=sl.tensor,
                offset=sl.offset + base + (dy0 - 1) * W + (dx0 - 1),
                ap=[sl.ap[0], [W, ndy], [1, ndx], *sl.ap[1:]],
            )

        def norm_chunk(ci, c0):
            st = stage_pool.tile([128, NK, CW], F32, name="st", tag="st")
            dmae = nc.sync if ci % 2 == 0 else nc.scalar
            dmae.dma_start(out=st[:], in_=av[:, :, c0 : c0 + CW])
            awc = aw[:, :, c0 : c0 + CW]
            nc.scalar.activation(
                out=awc, in_=st[:], func=mybir.ActivationFunctionType.Abs
            )
            s = psump.tile([128, CW], F32, name="s", tag="s")
            for k in range(NK):
                nc.tensor.matmul(
                    s[:],
                    idt[:],
                    aw[:, k, c0 : c0 + CW],
                    start=(k == 0),
                    stop=(k == NK - 1),
                )
            # 1/sum straight to fp16 (fast-recip spec, fp16 output AP)
            from concourse.dve_ops import (
                RECIP_APPROX_FAST_CONSTS,
                RECIPROCAL_APPROX_FAST,
            )

            r16 = small.tile([128, CW], F16, name="r16", tag="r16")
            c = RECIP_APPROX_FAST_CONSTS
            nc.vector._custom_dve(
                RECIPROCAL_APPROX_FAST,
                out=r16[:],
                in0=s[:],
                s0=c["s0"],
                s1=c["s1"],
                imm2=c["imm2"],
            )
            # rm rows (dx=-1,0,+1) = mask row * r; middle mask row is ones
            rm = small.tile([128, 3, CW], F16, name="rm", tag="rm")
            q0 = c0 % W
            nc.vector.tensor_mul(
                out=rm[:], in0=msk3[:, :, q0 : q0 + CW], in1=_bcast(r16[:], 3)
            )
            # fold mask * 1/sum into all 9 weight planes in one op
            awv = aw4(0, 3, 0, 3, c0, CW)
            rsl = rm[:]
            rmv = bass.AP(
                tensor=rsl.tensor, offset=rsl.offset, ap=[rsl.ap[0], [0, 3], *rsl.ap[1:]]
            )
            nc.vector.tensor_mul(out=awv, in0=awv, in1=rmv)

        def iter_chunk(t, c):
            fc, fn = fb[t % 2], fb[(t + 1) % 2]
            last = t == ITERS - 1
            c0 = c * CI
            base = HA + c0
            acc = psump.tile([128, CI], F32, name="acc", tag="acc")
            prod = prodp.tile([128, 3, 3, CI], F16, name="prod", tag="prod")
            groups = GROUPS[c]
            for eng, dy0, ndy, dx0, ndx in groups:
                out = prod[:, dy0 : dy0 + ndy, dx0 : dx0 + ndx, :]
                in0 = aw4(dy0, ndy, dx0, ndx, c0, CI)
                in1 = fview(fc, base, dy0, ndy, dx0, ndx, CI)
                e = nc.gpsimd if eng == "pool" else nc.vector
                e.tensor_mul(out=out, in0=in0, in1=in1)
            planes = []
            for gi in MM_ORDER[c]:
                _, dy0, ndy, dx0, ndx = groups[gi]
                planes += [
                    (dy, dx)
                    for dy in range(dy0, dy0 + ndy)
                    for dx in range(dx0, dx0 + ndx)
                ]
            for pi, (dy, dx) in enumerate(planes):
                for s0 in range(0, CI, 512):  # matmul free size capped at 512
                    se = min(s0 + 512, CI)
                    nc.tensor.matmul(
                        acc[:, s0:se],
                        idt[:],
                        prod[:, dy, dx, s0:se],
                        start=(pi == 0),
                        stop=(pi == NK - 1),
                    )
            if last:
                for q0 in range(0, CI, 510):
                    ost = outp.tile([128, 510], F32, name="ost", tag="ost")
                    nc.scalar.copy(out=ost[:], in_=acc[:, q0 : q0 + 510])
                    nc.sync.dma_start(
                        out=of[:, c0 + q0 : c0 + q0 + 510], in_=ost[:]
                    )
            else:
                nc.scalar.copy(out=fn[:, base : base + CI], in_=acc[:])
                # interior halo refresh by partition-shifted SBUF->SBUF DMA
                # (outermost halos stay at the zeros set by the PE refresh);
                # in phase 1 the DMA queues are busy with the affinity stream,
                # so iter 0 uses the PE refresh instead (emitted by caller).
                if t > 0 and c == 0:
                    nc.scalar.dma_start(
                        out=fn[0:127, HA + CH : FW], in_=fn[1:128, HA : 2 * HA]
                    )
                if t > 0 and c == 3:
                    nc.scalar.dma_start(
                        out=fn[1:128, 0:HA], in_=fn[0:127, CH : CH + HA]
                    )

        def refresh_pe(ft):
            """Full halo build on TensorE + ScalarE (phase 1 only).  The
            shift matrices' zero rows set the outermost halos to exactly 0 =
            the reference's dy zero padding."""
            phR = psump.tile([128, HA], F32, name="phR", tag="halo", bufs=1)
            for s0 in range(0, HA, 512):
                se = min(s0 + 512, HA)
                nc.tensor.matmul(
                    phR[:, s0:se], sup, ft[:, HA + s0 : HA + se], start=True, stop=True
                )
            nc.scalar.copy(out=ft[:, HA + CH : FW], in_=phR[:])
            phL = psump.tile([128, HA], F32, name="phL", tag="halo", bufs=1)
            for s0 in range(0, HA, 512):
                se = min(s0 + 512, HA)
                nc.tensor.matmul(
                    phL[:, s0:se], sdn, ft[:, CH + s0 : CH + se], start=True, stop=True
                )
            nc.scalar.copy(out=ft[:, 0:HA], in_=phL[:])

        # ---- schedule ----
        nc.sync.dma_start(out=idt3[:], in_=ident)
        nc.scalar.dma_start(out=msk3[:], in_=m)
        for ci in range(4):
            norm_chunk(ci, ci * CW)
            if ci < 2:  # feature load + fp32->fp16 convert on the idle Pool
                fst = stage_pool.tile([128, 2040], F32, name="fst", tag="fst")
                nc.sync.dma_start(out=fst[:], in_=ff[:, ci * 2040 : (ci + 1) * 2040])
                nc.gpsimd.tensor_copy(
                    out=fb[0][:, HA + ci * 2040 : HA + (ci + 1) * 2040], in_=fst[:]
                )
        refresh_pe(fb[0])
        iter_chunk(0, 0)
        for ci in range(4, 16):
            norm_chunk(ci, ci * CW)
            if ci % 4 == 3:
                iter_chunk(0, ci // 4)
        refresh_pe(fb[1])

        for t in range(1, ITERS):
            for c in range(4):
                iter_chunk(t, c)


def _masks():
    # msk3[p, row, col]: wrap-column masks at x = (240*(p%4) + col) mod W -
    # partition p starts at pixel 4080p and 4080 mod W = 240, so the
    # W-periodic masks have 4 partition phases.  Rows are dx = -1 (zero at
    # x==0), dx = 0 (ones), dx = +1 (zero at x==W-1).
    col = np.arange(MW)
    out = np.empty((128, 3, MW), np.float16)
    for ph in range(4):
        x = (240 * ph + col) % W
        out[ph::4, 0] = (x != 0).astype(np.float16)
        out[ph::4, 1] = 1.0
        out[ph::4, 2] = (x != W - 1).astype(np.float16)
    return out


def _get_nc():
    if "nc" not in _nc_cache:
        _nc_cache["nc"] = _build()
    return _nc_cache["nc"]


def _run(affinity, feature, **spmd_kwargs):
    affinity = np.ascontiguousarray(np.asarray(affinity, dtype=np.float32))
    feature = np.ascontiguousarray(np.asarray(feature, dtype=np.float32))
    nbatch = affinity.shape[0]
    nc = _get_nc()
    masks = _masks()
    ident = np.ascontiguousarray(
        np.stack(
            [
                np.eye(128, dtype=np.float16),
                np.eye(128, k=1, dtype=np.float16),
                np.eye(128, k=-1, dtype=np.float16),
            ]
        ).transpose(1, 0, 2)
    )
    in_maps = [
        {"a": affinity[i], "f": feature[i, 0], "m": masks, "ident": ident}
        for i in range(nbatch)
    ]
    res = run_bass_kernel_spmd(nc, in_maps, core_ids=list(range(nbatch)), **spmd_kwargs)
    out = np.stack([r["o"] for r in res.results])[:, None, :, :]
    return out.astype(np.float32), res


def kernel(affinity, feature):
    out, _ = _run(affinity, feature)
    return out
